# revision 24
# baseline (speedup 1.0000x reference)
"""GCN layer (BGRL-style) on 8 Trainium2 NeuronCores.

Math: the reference computes
  log_softmax(relu((A_hat @ (X*norm_src)) @ W_conv * norm_dst + b) @ W2 + b2).
Aggregation is linear and in_feats > hidden, so each core first computes
h = (X*norm_src) @ W_conv for ALL nodes (redundantly, avoiding collectives)
into DRAM h-tables (bf16, 512B rows), then aggregates h[src] per
destination block — halving the per-edge gather traffic vs gathering raw
features.

The h-tables are split by node block: h_lo holds nodes with n//128 < 136,
h_hi the rest (both <= 32768 rows, the dma_gather int16 index limit), each
PARTITION-MAJOR (node n at row (n%128)*NBLKS + n//128 - base) so phase-1
stores are one 4KB-contiguous-per-partition DMA per 8-block batch. Because
h_lo completes at the phase-1 midpoint, the lo-side gathers AND the lo half
of every block's aggregation run concurrently with the hi half of phase 1;
the partial sums are spilled to SBUF (bf16) and added back in the hi pass.

Sharding: dst nodes are greedily assigned to 8*49 groups of <=128 slots,
jointly balancing each group's lo- and hi-edge counts; the host unpermutes
output rows at the end.

Per 128-dst block, fully on-chip:
  - h[src] rows arrive via dma_gather (SWDGE ucode, ~1us fixed +
    0.34ns/row) landing [128, T, 256] tiles in (lane=i%128, tile=i//128)
    edge order,
  - segment-sums TRANSPOSED via one-hot S matmuls (gathered h-chunks as
    lhsT, S as rhs) -> xT [h x d] in PSUM, so the downstream chain needs no
    transposes: free-dim broadcast multiply by norm_dst, relu+bias
    (per-partition, h on partitions), W2 matmuls, bias outer-products via
    K=1 matmuls, and log_softmax, streaming fp32 logits out per block.
All 8 cores run one SPMD program; edge partitions are padded to uniform
tiles-per-block counts T_LO/T_HI (pad lanes get idx 0 + sentinel dst 255,
whose S column is all-zero).
"""

import numpy as np

N = 50000
F = 512
H = 256
C = 64
P = 8
NB = 49                  # dst blocks per core
NG = P * NB              # 392 dst groups, <=128 nodes each
NPC = NB * 128           # 6272 output rows per core (incl. pad slots)
NBLK = 392               # node blocks for phase 1 (392*128 = 50176 >= N)
NPAD = NBLK * 128
LOBLK = 136              # node blocks in h_lo (17408 rows); small so h_lo
                         # finishes early and lo gathers overlap phase 1
HIBLK = NBLK - LOBLK     # 256 node blocks in h_hi (32768 rows = int16 max)
NLO = LOBLK * 128
NHI = HIBLK * 128
GB = 8                   # node blocks per phase-1 batch
NGRP1 = NBLK // GB       # 49 phase-1 batches (24 lo + 25 hi)
NQ = 4                   # SWDGE queues

_cache = {}
_trace = False          # set by test harness for profiled runs
_trace_tmpdir = None
_last_results = None


def _build_program(T_LO, T_HI, bench_R=0):
    import concourse.mybir as mybir
    import concourse.tile as tile
    from concourse import bacc, library_config
    from concourse.tile_rust import add_dep_helper

    dt = mybir.dt

    nc = bacc.Bacc("TRN2", target_bir_lowering=False, debug=False,
                   num_devices=P, num_swdge_queues=NQ)

    featT_d = nc.dram_tensor("featT", [128, NBLK * 512], dt.bfloat16,
                             kind="ExternalInput")
    h_lo_d = nc.dram_tensor("h_lo", [NLO, H], dt.bfloat16, kind="Internal")
    h_hi_d = nc.dram_tensor("h_hi", [NHI, H], dt.bfloat16, kind="Internal")
    ixlo_d = nc.dram_tensor("ixlo", [128, NB * T_LO * 8], dt.int16,
                            kind="ExternalInput")
    ixhi_d = nc.dram_tensor("ixhi", [128, NB * T_HI * 8], dt.int16,
                            kind="ExternalInput")
    dllo_d = nc.dram_tensor("dllo", [128, NB * T_LO], dt.bfloat16,
                            kind="ExternalInput")
    dlhi_d = nc.dram_tensor("dlhi", [128, NB * T_HI], dt.bfloat16,
                            kind="ExternalInput")
    # normdst materialized across partitions (DVE lanes cannot broadcast
    # along the partition dim): every row identical
    normdst_d = nc.dram_tensor("normdst", [128, NB * 128], dt.bfloat16,
                               kind="ExternalInput")
    iota_d = nc.dram_tensor("iota", [128, 128], dt.bfloat16, kind="ExternalInput")
    wconv_d = nc.dram_tensor("wconv", [128, 4 * H], dt.bfloat16,
                             kind="ExternalInput")
    w2_d = nc.dram_tensor("w2", [128, 2 * C], dt.bfloat16, kind="ExternalInput")
    ones_d = nc.dram_tensor("ones1", [1, 128], dt.bfloat16, kind="ExternalInput")
    # bconv as [128, 2] (h on partitions, one col per h-half)
    bconv_d = nc.dram_tensor("bconv", [128, 2], dt.float32, kind="ExternalInput")
    b2_d = nc.dram_tensor("b2r", [1, C], dt.bfloat16, kind="ExternalInput")
    out_d = nc.dram_tensor("out", [NPC, C], dt.float32, kind="ExternalOutput")

    h_lo_pm = h_lo_d[:].rearrange("(p n) c -> p n c", p=128)
    h_hi_pm = h_hi_d[:].rearrange("(p n) c -> p n c", p=128)

    with tile.TileContext(nc) as tc:
        with (
            tc.tile_pool(name="const", bufs=1) as cpool,
            tc.tile_pool(name="x1", bufs=2) as xpool,
            tc.tile_pool(name="h1", bufs=2) as hpool,
            tc.tile_pool(name="work", bufs=3) as wpool,
            tc.tile_pool(name="gath", bufs=7) as gpool,
            tc.tile_pool(name="psA", bufs=3, space="PSUM") as ppool,
            tc.tile_pool(name="psB", bufs=2, space="PSUM") as ppool1,
        ):
            lib = nc.gpsimd.load_library(library_config.mlp)

            # --- constants / metadata, loaded once ---
            iota_t = cpool.tile([128, 128], dt.bfloat16, tag="iota")
            nc.sync.dma_start(iota_t[:], iota_d[:])
            wconv_t = cpool.tile([128, 4 * H], dt.bfloat16, tag="wconv")
            nc.sync.dma_start(wconv_t[:], wconv_d[:])
            w2_t = cpool.tile([128, 2 * C], dt.bfloat16, tag="w2")
            nc.sync.dma_start(w2_t[:], w2_d[:])
            ones_t = cpool.tile([1, 128], dt.bfloat16, tag="ones")
            nc.sync.dma_start(ones_t[:], ones_d[:])
            bconv_t = cpool.tile([128, 2], dt.float32, tag="bconv")
            nc.sync.dma_start(bconv_t[:], bconv_d[:])
            b2_t = cpool.tile([1, C], dt.bfloat16, tag="b2")
            nc.sync.dma_start(b2_t[:], b2_d[:])
            ixlo_t = cpool.tile([128, NB * T_LO * 8], dt.int16, tag="ixlo")
            nc.sync.dma_start(ixlo_t[:], ixlo_d[:])
            ixhi_t = cpool.tile([128, NB * T_HI * 8], dt.int16, tag="ixhi")
            nc.sync.dma_start(ixhi_t[:], ixhi_d[:])
            dllo_t = cpool.tile([128, NB * T_LO], dt.bfloat16, tag="dllo")
            nc.sync.dma_start(dllo_t[:], dllo_d[:])
            dlhi_t = cpool.tile([128, NB * T_HI], dt.bfloat16, tag="dlhi")
            nc.sync.dma_start(dlhi_t[:], dlhi_d[:])
            normdst_t = cpool.tile([128, NB * 128], dt.bfloat16, tag="normdst")
            nc.sync.dma_start(normdst_t[:], normdst_d[:])
            # spilled lo partial sums, one [128, 256] bf16 slot per block
            xlo_all = cpool.tile([128, NB, H], dt.bfloat16, tag="xlo")

            iota_lo = iota_t[:].rearrange("p (o n) -> p o n", o=1).broadcast_to(
                [128, T_LO, 128])
            iota_hi = iota_t[:].rearrange("p (o n) -> p o n", o=1).broadcast_to(
                [128, T_HI, 128])

            def phase1(gi):
                xt = xpool.tile([128, GB, 512], dt.bfloat16, tag="xt")
                nc.sync.dma_start(
                    xt[:], featT_d[:, gi * GB * 512:(gi + 1) * GB * 512]
                    .rearrange("p (t f) -> p t f", t=GB))
                hsb = hpool.tile([128, GB, H], dt.bfloat16, tag="hsb")
                for t in range(GB):
                    hps = ppool.tile([128, H], dt.float32, tag="hps")
                    for c in range(4):
                        nc.tensor.matmul(
                            hps[:],
                            xt[:, t, c * 128:(c + 1) * 128],
                            wconv_t[:, c * H:(c + 1) * H],
                            start=(c == 0), stop=(c == 3),
                        )
                    if t % 2 == 0:
                        nc.vector.tensor_copy(hsb[:, t, :], hps[:])
                    else:
                        nc.scalar.activation(
                            hsb[:, t, :], hps[:],
                            mybir.ActivationFunctionType.Copy)
                # node (gi*8+t)*128 + p -> row p*NBLKS + (gi*8+t) - base
                if gi < LOBLK // GB:
                    dst = h_lo_pm[:, gi * GB:(gi + 1) * GB, :]
                else:
                    gj = gi - LOBLK // GB
                    dst = h_hi_pm[:, gj * GB:(gj + 1) * GB, :]
                nc.scalar.dma_start(dst, hsb[:])

            def agg_pass(b, T_X, iota_x, dl_t, ix_t, h_d, lo):
                # S one-hot: S[p, t*128+j] = (dl[p, b*T_X+t] == j)
                tag = "Sl" if lo else "Sh"
                S = wpool.tile([128, T_X, 128], dt.bfloat16, tag=tag,
                               bufs=3 if lo else 2)
                nc.vector.tensor_tensor(
                    S[:], iota_x,
                    dl_t[:, b * T_X:(b + 1) * T_X].broadcast_to(
                        [128, T_X, 128]),
                    op=mybir.AluOpType.is_equal,
                )
                # shared tag across lo/hi passes: 7 rotating bufs keep ~7
                # gathers in flight (random 512B HBM reads need deep queues);
                # two half-gathers per block on different queues so at least
                # two SWDGE rings drain concurrently
                g = gpool.tile([128, T_X, H], dt.bfloat16, tag="g")
                TA = (T_X + 1) // 2
                ixb = b * T_X * 8
                g1 = nc.gpsimd.dma_gather(
                    g[:, 0:TA, :], h_d[:], ix_t[:, ixb:ixb + TA * 8],
                    TA * 128, TA * 128, H,
                    queue_num=(2 * b) % NQ, single_packet=False)
                add_dep_helper(g1.ins, lib.ins, reason="lib first")
                g2 = nc.gpsimd.dma_gather(
                    g[:, TA:T_X, :], h_d[:],
                    ix_t[:, ixb + TA * 8:ixb + T_X * 8],
                    (T_X - TA) * 128, (T_X - TA) * 128, H,
                    queue_num=(2 * b + 1) % NQ, single_packet=False)
                add_dep_helper(g2.ins, lib.ins, reason="lib first")
                # xT[half] [128h x 128d] += g[:,t,half].T @ S[:,t,:]
                xtp = ppool1.tile([128, H], dt.float32,
                                  tag="xtpa" if lo else "xtpb")
                for t in range(T_X):
                    for half in range(2):
                        nc.tensor.matmul(
                            xtp[:, half * 128:(half + 1) * 128],
                            g[:, t, half * 128:(half + 1) * 128],
                            S[:, t, :],
                            start=(t == 0), stop=(t == T_X - 1),
                        )
                return xtp

            def head(b, xtp):
                # x = relu((xtp + xlo) * norm_dst[d] + b_conv[h]); norm_dst
                # along free dim (d), bias per partition (h)
                xs = wpool.tile([128, H], dt.float32, tag="xs")
                nc.vector.tensor_tensor(xs[:], xtp[:], xlo_all[:, b, :],
                                        op=mybir.AluOpType.add)
                xn = wpool.tile([128, H], dt.float32, tag="xn")
                nc.vector.tensor_tensor(
                    xn[:].rearrange("p (o n) -> p o n", o=2),
                    xs[:].rearrange("p (o n) -> p o n", o=2),
                    normdst_t[:, b * 128:(b + 1) * 128]
                    .rearrange("p (o n) -> p o n", o=1)
                    .broadcast_to([128, 2, 128]),
                    op=mybir.AluOpType.mult,
                )
                xts = wpool.tile([128, H], dt.bfloat16, tag="xts")
                for half in range(2):
                    nc.scalar.activation(
                        xts[:, half * 128:(half + 1) * 128],
                        xn[:, half * 128:(half + 1) * 128],
                        mybir.ActivationFunctionType.Relu,
                        bias=bconv_t[:, half:half + 1],
                    )
                # logits [128d x 64] = sum_half xts[half].T @ w2[half] + b2
                lps = ppool1.tile([128, C], dt.float32, tag="lps", bufs=1)
                for half in range(2):
                    nc.tensor.matmul(
                        lps[:], xts[:, half * 128:(half + 1) * 128],
                        w2_t[:, half * C:(half + 1) * C],
                        start=(half == 0), stop=False,
                    )
                nc.tensor.matmul(lps[:], ones_t[:], b2_t[:],
                                 start=False, stop=True)
                # log_softmax along classes
                mneg = wpool.tile([128, 1], dt.float32, tag="mneg")
                nc.vector.reduce_max(mneg[:], lps[:],
                                     axis=mybir.AxisListType.X, negate=True)
                esc = wpool.tile([128, C], dt.float32, tag="esc")
                ssum = wpool.tile([128, 1], dt.float32, tag="ssum")
                nc.scalar.activation(
                    esc[:], lps[:], mybir.ActivationFunctionType.Exp,
                    bias=mneg[:], accum_out=ssum[:],
                )
                lse = wpool.tile([128, 1], dt.float32, tag="lse")
                nc.scalar.activation(lse[:], ssum[:],
                                     mybir.ActivationFunctionType.Ln)
                shift = wpool.tile([128, 1], dt.float32, tag="shift")
                nc.vector.tensor_tensor(shift[:], mneg[:], lse[:],
                                        op=mybir.AluOpType.subtract)
                osb = wpool.tile([128, C], dt.float32, tag="osb")
                nc.vector.tensor_scalar_add(osb[:], lps[:], shift[:])
                nc.sync.dma_start(out_d[b * 128:(b + 1) * 128, :], osb[:])

            def body():
                # phase 1 lo: h_lo complete after 24 batches
                for gi in range(LOBLK // GB):
                    phase1(gi)
                # phase 1 hi + (scheduler-overlapped) lo gathers/aggs
                for gi in range(LOBLK // GB, NGRP1):
                    phase1(gi)
                for b in range(NB):
                    xtp = agg_pass(b, T_LO, iota_lo, dllo_t, ixlo_t,
                                   h_lo_d, lo=True)
                    # spill on Scalar: Vector is loaded with S-builds here
                    nc.scalar.activation(xlo_all[:, b, :], xtp[:],
                                         mybir.ActivationFunctionType.Copy)
                for b in range(NB):
                    xtp = agg_pass(b, T_HI, iota_hi, dlhi_t, ixhi_t,
                                   h_hi_d, lo=False)
                    head(b, xtp)

            if bench_R:
                with tc.For_i(0, bench_R, 1):
                    body()
            else:
                body()

    nc.compile()
    return nc


def _balance_groups(deg_lo, deg_hi):
    """Greedily assign nodes to NG groups (<=128 each), jointly balancing
    lo- and hi-edge sums (normalized by their global masses, which differ
    when the lo/hi table split is asymmetric).

    Returns (grp_of, slot_of) int32 arrays of length N."""
    w_lo = NG / max(deg_lo.sum(), 1.0)
    w_hi = NG / max(deg_hi.sum(), 1.0)
    order = np.argsort(-(deg_lo + deg_hi), kind="stable")
    lo_s = np.zeros(NG, np.float64)
    hi_s = np.zeros(NG, np.float64)
    sizes = np.zeros(NG, np.int64)
    grp_of = np.empty(N, np.int32)
    slot_of = np.empty(N, np.int32)
    full = np.zeros(NG, bool)
    BIG = 1e18
    for n in order.tolist():
        cost = np.maximum((lo_s + deg_lo[n]) * w_lo,
                          (hi_s + deg_hi[n]) * w_hi)
        cost[full] = BIG
        g = int(np.argmin(cost))
        grp_of[n] = g
        slot_of[n] = sizes[g]
        sizes[g] += 1
        if sizes[g] == 128:
            full[g] = True
        lo_s[g] += deg_lo[n]
        hi_s[g] += deg_hi[n]
    return grp_of, slot_of


def _pack_idx(vals):
    """[T*128] int16 -> [128, T*8] device idx layout.

    dma_gather reads idx i from [i % 16, i // 16] of a [16, NI/16] block,
    replicated across the 8 gpsimd core groups."""
    a = vals.reshape(-1, 16).T          # [16, NI/16]
    return np.tile(a, (8, 1))


def _prep(features, W_conv, b_conv, W2, b2, src, dst):
    import ml_dtypes
    bf16 = ml_dtypes.bfloat16

    src = np.asarray(src).astype(np.int64)
    dst = np.asarray(dst).astype(np.int64)
    deg_out = np.bincount(src, minlength=N).astype(np.float32)
    deg_in = np.bincount(dst, minlength=N).astype(np.float32)
    norm_src = 1.0 / np.sqrt(deg_out)
    norm_dst = 1.0 / np.sqrt(deg_in)

    # h-table row of node s: (s%128)*NBLKS + s//128 - base; h_lo holds node
    # blocks < LOBLK, h_hi the rest
    s_nb = src // 128
    s_p = src % 128
    elo = s_nb < LOBLK
    s_row = np.where(elo, s_p * LOBLK + s_nb, s_p * HIBLK + (s_nb - LOBLK))

    # per-dst lo/hi in-degrees for the 2D balance
    dlo = np.bincount(dst[elo], minlength=N).astype(np.float32)
    dhi = deg_in - dlo
    grp_of, slot_of = _balance_groups(dlo, dhi)

    # normalized features, transposed + blocked for phase 1:
    # featT[p, (nb*4 + c)*128 + j] = Xn[nb*128 + j, c*128 + p]
    feat_n = (np.asarray(features, np.float32) * norm_src[:, None]).astype(bf16)
    xp = np.zeros((NPAD, F), bf16)
    xp[:N] = feat_n
    featT = np.ascontiguousarray(
        xp.reshape(NBLK, 128, 4, 128).transpose(3, 0, 2, 1)
    ).reshape(128, NBLK * 512)

    # edges -> (group, lo/hi, tile, lane)
    eg = grp_of[dst]
    cnt_lo = np.bincount(eg[elo], minlength=NG)
    cnt_hi = np.bincount(eg[~elo], minlength=NG)
    T_LO = int(np.ceil(cnt_lo.max() / 128))
    T_HI = int(np.ceil(cnt_hi.max() / 128))

    dloc = slot_of[dst].astype(np.float32)

    def layout(mask, cnts, L):
        r_e = s_row[mask]
        g_e = eg[mask]
        d_e = dloc[mask]
        # sort by (group, table row) so each tile's rows ascend in the table
        order = np.lexsort((r_e, g_e))
        g_s = g_e[order]
        starts = np.zeros(NG + 1, np.int64)
        np.cumsum(cnts, out=starts[1:])
        pos = np.arange(len(g_s)) - starts[g_s]
        slot = g_s * L + pos
        idx_pad = np.zeros(NG * L, np.int16)
        dl_pad = np.full(NG * L, 255.0, np.float32)
        idx_pad[slot] = r_e[order].astype(np.int16)
        dl_pad[slot] = d_e[order]
        return idx_pad.reshape(NG, L), dl_pad.reshape(NG, L)

    idx_lo, dl_lo = layout(elo, cnt_lo, T_LO * 128)
    idx_hi, dl_hi = layout(~elo, cnt_hi, T_HI * 128)

    # normdst per core: [NB*128] values in permuted slot order, pad 1.0
    nd = np.ones(NG * 128, np.float32)
    nd[grp_of * 128 + slot_of] = norm_dst

    iota = np.broadcast_to(np.arange(128, dtype=np.float32), (128, 128)).astype(bf16)
    wconv = np.ascontiguousarray(
        np.asarray(W_conv, np.float32).reshape(4, 128, H).transpose(1, 0, 2)
    ).reshape(128, 4 * H).astype(bf16)
    w2r = np.ascontiguousarray(
        np.asarray(W2, np.float32).reshape(2, 128, C).transpose(1, 0, 2)
    ).reshape(128, 2 * C).astype(bf16)

    in_maps = []
    for c in range(P):
        gsl = slice(c * NB, (c + 1) * NB)
        # dstloc per core+split: [128, NB*T_X]; tile column (b*T_X + t),
        # lane p holds the slot of edge (tile t of block b, lane p)
        dllo = np.ascontiguousarray(
            dl_lo[gsl].reshape(NB * T_LO, 128).T).astype(bf16)
        dlhi = np.ascontiguousarray(
            dl_hi[gsl].reshape(NB * T_HI, 128).T).astype(bf16)
        ixlo = np.concatenate(
            [_pack_idx(idx_lo[g]) for g in range(c * NB, (c + 1) * NB)],
            axis=1)
        ixhi = np.concatenate(
            [_pack_idx(idx_hi[g]) for g in range(c * NB, (c + 1) * NB)],
            axis=1)
        in_maps.append({
            "featT": featT,
            "ixlo": np.ascontiguousarray(ixlo),
            "ixhi": np.ascontiguousarray(ixhi),
            "dllo": dllo,
            "dlhi": dlhi,
            "normdst": np.ascontiguousarray(np.broadcast_to(
                nd[c * NPC:(c + 1) * NPC].astype(bf16), (128, NB * 128))),
            "iota": iota,
            "wconv": wconv,
            "w2": w2r,
            "ones1": np.ones((1, 128), np.float32).astype(bf16),
            "bconv": np.asarray(b_conv, np.float32).reshape(2, 128).T.copy(),
            "b2r": np.asarray(b2, np.float32).reshape(1, C).astype(bf16),
        })
    return T_LO, T_HI, grp_of, slot_of, in_maps


def kernel(features, W_conv, b_conv, W2, b2, src, dst):
    from concourse.bass_utils import run_bass_kernel_spmd

    T_LO, T_HI, grp_of, slot_of, in_maps = _prep(
        features, W_conv, b_conv, W2, b2, src, dst)
    key = (T_LO, T_HI)
    if key not in _cache:
        _cache[key] = _build_program(T_LO, T_HI)
    nc = _cache[key]
    res = run_bass_kernel_spmd(nc, in_maps, core_ids=list(range(P)),
                               trace=_trace, tmpdir=_trace_tmpdir)
    global _last_results
    _last_results = res
    rows = np.concatenate([res.results[c]["out"] for c in range(P)], axis=0)
    out = rows[grp_of * 128 + slot_of]
    return out.astype(np.float32)


# revision 26
# speedup vs baseline: 1.0411x; 1.0411x over previous
"""GCN layer (BGRL-style) on 8 Trainium2 NeuronCores.

Math: the reference computes
  log_softmax(relu((A_hat @ (X*norm_src)) @ W_conv * norm_dst + b) @ W2 + b2).
Aggregation is linear and in_feats > hidden, so each core first computes
h = (X*norm_src) @ W_conv for ALL nodes (redundantly, avoiding collectives)
into DRAM h-tables (bf16, 512B rows), then aggregates h[src] per
destination block — halving the per-edge gather traffic vs gathering raw
features.

The h-tables are split by node block: h_lo holds nodes with n//128 < 136,
h_hi the rest (both <= 32768 rows, the dma_gather int16 index limit), each
PARTITION-MAJOR (node n at row (n%128)*NBLKS + n//128 - base) so phase-1
stores are one 4KB-contiguous-per-partition DMA per 8-block batch. Because
h_lo completes at the phase-1 midpoint, the lo-side gathers AND the lo half
of every block's aggregation run concurrently with the hi half of phase 1;
the partial sums are spilled to SBUF (bf16) and added back in the hi pass.

Sharding: dst nodes are greedily assigned to 8*49 groups of <=128 slots,
jointly balancing each group's lo- and hi-edge counts; the host unpermutes
output rows at the end.

Per 128-dst block, fully on-chip:
  - h[src] rows arrive via dma_gather (SWDGE ucode, ~1us fixed +
    0.34ns/row) landing [128, T, 256] tiles in (lane=i%128, tile=i//128)
    edge order,
  - segment-sums TRANSPOSED via one-hot S matmuls (gathered h-chunks as
    lhsT, S as rhs) -> xT [h x d] in PSUM, so the downstream chain needs no
    transposes: free-dim broadcast multiply by norm_dst, relu+bias
    (per-partition, h on partitions), W2 matmuls, bias outer-products via
    K=1 matmuls, and log_softmax, streaming fp32 logits out per block.
All 8 cores run one SPMD program; edge partitions are padded to uniform
tiles-per-block counts T_LO/T_HI (pad lanes get idx 0 + sentinel dst 255,
whose S column is all-zero).
"""

import numpy as np

N = 50000
F = 512
H = 256
C = 64
P = 8
NB = 49                  # dst blocks per core
NG = P * NB              # 392 dst groups, <=128 nodes each
NPC = NB * 128           # 6272 output rows per core (incl. pad slots)
NBLK = 392               # node blocks for phase 1 (392*128 = 50176 >= N)
NPAD = NBLK * 128
LOBLK = 136              # node blocks in h_lo (17408 rows); small so h_lo
                         # finishes early and lo gathers overlap phase 1
HIBLK = NBLK - LOBLK     # 256 node blocks in h_hi (32768 rows = int16 max)
NLO = LOBLK * 128
NHI = HIBLK * 128
GB = 8                   # node blocks per phase-1 batch
NGRP1 = NBLK // GB       # 49 phase-1 batches (24 lo + 25 hi)
NQ = 4                   # SWDGE queues

_cache = {}
_trace = False          # set by test harness for profiled runs
_trace_tmpdir = None
_last_results = None


def _build_program(T_LO, T_HI, bench_R=0):
    import concourse.mybir as mybir
    import concourse.tile as tile
    from concourse import bacc, library_config
    from concourse.tile_rust import add_dep_helper

    dt = mybir.dt

    nc = bacc.Bacc("TRN2", target_bir_lowering=False, debug=False,
                   num_devices=P, num_swdge_queues=NQ)

    featT_d = nc.dram_tensor("featT", [128, NBLK * 512], dt.bfloat16,
                             kind="ExternalInput")
    h_lo_d = nc.dram_tensor("h_lo", [NLO, H], dt.bfloat16, kind="Internal")
    h_hi_d = nc.dram_tensor("h_hi", [NHI, H], dt.bfloat16, kind="Internal")
    ixlo_d = nc.dram_tensor("ixlo", [128, NB * T_LO * 8], dt.int16,
                            kind="ExternalInput")
    ixhi_d = nc.dram_tensor("ixhi", [128, NB * T_HI * 8], dt.int16,
                            kind="ExternalInput")
    dllo_d = nc.dram_tensor("dllo", [128, NB * T_LO], dt.bfloat16,
                            kind="ExternalInput")
    dlhi_d = nc.dram_tensor("dlhi", [128, NB * T_HI], dt.bfloat16,
                            kind="ExternalInput")
    # normdst materialized across partitions (DVE lanes cannot broadcast
    # along the partition dim): every row identical
    normdst_d = nc.dram_tensor("normdst", [128, NB * 128], dt.bfloat16,
                               kind="ExternalInput")
    iota_d = nc.dram_tensor("iota", [128, 128], dt.bfloat16, kind="ExternalInput")
    wconv_d = nc.dram_tensor("wconv", [128, 4 * H], dt.bfloat16,
                             kind="ExternalInput")
    w2_d = nc.dram_tensor("w2", [128, 2 * C], dt.bfloat16, kind="ExternalInput")
    ones_d = nc.dram_tensor("ones1", [1, 128], dt.bfloat16, kind="ExternalInput")
    # bconv as [128, 2] (h on partitions, one col per h-half)
    bconv_d = nc.dram_tensor("bconv", [128, 2], dt.float32, kind="ExternalInput")
    b2_d = nc.dram_tensor("b2r", [1, C], dt.bfloat16, kind="ExternalInput")
    out_d = nc.dram_tensor("out", [NPC, C], dt.float32, kind="ExternalOutput")

    h_lo_pm = h_lo_d[:].rearrange("(p n) c -> p n c", p=128)
    h_hi_pm = h_hi_d[:].rearrange("(p n) c -> p n c", p=128)

    with tile.TileContext(nc) as tc:
        with (
            tc.tile_pool(name="const", bufs=1) as cpool,
            tc.tile_pool(name="x1", bufs=2) as xpool,
            tc.tile_pool(name="h1", bufs=2) as hpool,
            tc.tile_pool(name="work", bufs=3) as wpool,
            tc.tile_pool(name="gath", bufs=7) as gpool,
            tc.tile_pool(name="psA", bufs=3, space="PSUM") as ppool,
            tc.tile_pool(name="psB", bufs=2, space="PSUM") as ppool1,
        ):
            lib = nc.gpsimd.load_library(library_config.mlp)

            # --- constants / metadata, loaded once ---
            iota_t = cpool.tile([128, 128], dt.bfloat16, tag="iota")
            nc.sync.dma_start(iota_t[:], iota_d[:])
            wconv_t = cpool.tile([128, 4 * H], dt.bfloat16, tag="wconv")
            nc.sync.dma_start(wconv_t[:], wconv_d[:])
            w2_t = cpool.tile([128, 2 * C], dt.bfloat16, tag="w2")
            nc.sync.dma_start(w2_t[:], w2_d[:])
            ones_t = cpool.tile([1, 128], dt.bfloat16, tag="ones")
            nc.sync.dma_start(ones_t[:], ones_d[:])
            bconv_t = cpool.tile([128, 2], dt.float32, tag="bconv")
            nc.sync.dma_start(bconv_t[:], bconv_d[:])
            b2_t = cpool.tile([1, C], dt.bfloat16, tag="b2")
            nc.sync.dma_start(b2_t[:], b2_d[:])
            ixlo_t = cpool.tile([128, NB * T_LO * 8], dt.int16, tag="ixlo")
            nc.sync.dma_start(ixlo_t[:], ixlo_d[:])
            ixhi_t = cpool.tile([128, NB * T_HI * 8], dt.int16, tag="ixhi")
            nc.sync.dma_start(ixhi_t[:], ixhi_d[:])
            dllo_t = cpool.tile([128, NB * T_LO], dt.bfloat16, tag="dllo")
            nc.sync.dma_start(dllo_t[:], dllo_d[:])
            dlhi_t = cpool.tile([128, NB * T_HI], dt.bfloat16, tag="dlhi")
            nc.sync.dma_start(dlhi_t[:], dlhi_d[:])
            normdst_t = cpool.tile([128, NB * 128], dt.bfloat16, tag="normdst")
            nc.sync.dma_start(normdst_t[:], normdst_d[:])
            # spilled lo partial sums, one [128, 256] bf16 slot per block
            xlo_all = cpool.tile([128, NB, H], dt.bfloat16, tag="xlo")

            iota_lo = iota_t[:].rearrange("p (o n) -> p o n", o=1).broadcast_to(
                [128, T_LO, 128])
            iota_hi = iota_t[:].rearrange("p (o n) -> p o n", o=1).broadcast_to(
                [128, T_HI, 128])

            def phase1(gi):
                xt = xpool.tile([128, GB, 512], dt.bfloat16, tag="xt")
                nc.sync.dma_start(
                    xt[:], featT_d[:, gi * GB * 512:(gi + 1) * GB * 512]
                    .rearrange("p (t f) -> p t f", t=GB))
                hsb = hpool.tile([128, GB, H], dt.bfloat16, tag="hsb")
                for t in range(GB):
                    hps = ppool.tile([128, H], dt.float32, tag="hps")
                    for c in range(4):
                        nc.tensor.matmul(
                            hps[:],
                            xt[:, t, c * 128:(c + 1) * 128],
                            wconv_t[:, c * H:(c + 1) * H],
                            start=(c == 0), stop=(c == 3),
                        )
                    if t % 2 == 0:
                        nc.vector.tensor_copy(hsb[:, t, :], hps[:])
                    else:
                        nc.scalar.activation(
                            hsb[:, t, :], hps[:],
                            mybir.ActivationFunctionType.Copy)
                # node (gi*8+t)*128 + p -> row p*NBLKS + (gi*8+t) - base
                if gi < LOBLK // GB:
                    dst = h_lo_pm[:, gi * GB:(gi + 1) * GB, :]
                else:
                    gj = gi - LOBLK // GB
                    dst = h_hi_pm[:, gj * GB:(gj + 1) * GB, :]
                nc.scalar.dma_start(dst, hsb[:])

            def agg_pass(b, T_X, iota_x, dl_t, ix_t, h_d, lo):
                # S one-hot: S[p, t*128+j] = (dl[p, b*T_X+t] == j)
                tag = "Sl" if lo else "Sh"
                S = wpool.tile([128, T_X, 128], dt.bfloat16, tag=tag,
                               bufs=3 if lo else 2)
                nc.vector.tensor_tensor(
                    S[:], iota_x,
                    dl_t[:, b * T_X:(b + 1) * T_X].broadcast_to(
                        [128, T_X, 128]),
                    op=mybir.AluOpType.is_equal,
                )
                # shared tag across lo/hi passes: 7 rotating bufs keep ~7
                # gathers in flight (random 512B HBM reads need deep queues);
                # two half-gathers per block on different queues so at least
                # two SWDGE rings drain concurrently
                g = gpool.tile([128, T_X, H], dt.bfloat16, tag="g")
                TA = (T_X + 1) // 2
                ixb = b * T_X * 8
                # hi pass offset by 2 queues so the lo-pass tail and hi-pass
                # head drain on disjoint rings at the transition
                qo = 0 if lo else 2
                g1 = nc.gpsimd.dma_gather(
                    g[:, 0:TA, :], h_d[:], ix_t[:, ixb:ixb + TA * 8],
                    TA * 128, TA * 128, H,
                    queue_num=(2 * b + qo) % NQ, single_packet=False)
                add_dep_helper(g1.ins, lib.ins, reason="lib first")
                g2 = nc.gpsimd.dma_gather(
                    g[:, TA:T_X, :], h_d[:],
                    ix_t[:, ixb + TA * 8:ixb + T_X * 8],
                    (T_X - TA) * 128, (T_X - TA) * 128, H,
                    queue_num=(2 * b + 1 + qo) % NQ, single_packet=False)
                add_dep_helper(g2.ins, lib.ins, reason="lib first")
                # xT[half] [128h x 128d] += g[:,t,half].T @ S[:,t,:]
                xtp = ppool1.tile([128, H], dt.float32,
                                  tag="xtpa" if lo else "xtpb")
                for t in range(T_X):
                    for half in range(2):
                        nc.tensor.matmul(
                            xtp[:, half * 128:(half + 1) * 128],
                            g[:, t, half * 128:(half + 1) * 128],
                            S[:, t, :],
                            start=(t == 0), stop=(t == T_X - 1),
                        )
                return xtp

            def head(b, xtp):
                # x = relu((xtp + xlo) * norm_dst[d] + b_conv[h]); norm_dst
                # along free dim (d), bias per partition (h)
                xs = wpool.tile([128, H], dt.float32, tag="xs")
                nc.vector.tensor_tensor(xs[:], xtp[:], xlo_all[:, b, :],
                                        op=mybir.AluOpType.add)
                xn = wpool.tile([128, H], dt.float32, tag="xn")
                nc.vector.tensor_tensor(
                    xn[:].rearrange("p (o n) -> p o n", o=2),
                    xs[:].rearrange("p (o n) -> p o n", o=2),
                    normdst_t[:, b * 128:(b + 1) * 128]
                    .rearrange("p (o n) -> p o n", o=1)
                    .broadcast_to([128, 2, 128]),
                    op=mybir.AluOpType.mult,
                )
                xts = wpool.tile([128, H], dt.bfloat16, tag="xts")
                for half in range(2):
                    nc.scalar.activation(
                        xts[:, half * 128:(half + 1) * 128],
                        xn[:, half * 128:(half + 1) * 128],
                        mybir.ActivationFunctionType.Relu,
                        bias=bconv_t[:, half:half + 1],
                    )
                # logits [128d x 64] = sum_half xts[half].T @ w2[half] + b2
                lps = ppool1.tile([128, C], dt.float32, tag="lps", bufs=1)
                for half in range(2):
                    nc.tensor.matmul(
                        lps[:], xts[:, half * 128:(half + 1) * 128],
                        w2_t[:, half * C:(half + 1) * C],
                        start=(half == 0), stop=False,
                    )
                nc.tensor.matmul(lps[:], ones_t[:], b2_t[:],
                                 start=False, stop=True)
                # log_softmax along classes
                mneg = wpool.tile([128, 1], dt.float32, tag="mneg")
                nc.vector.reduce_max(mneg[:], lps[:],
                                     axis=mybir.AxisListType.X, negate=True)
                esc = wpool.tile([128, C], dt.float32, tag="esc")
                ssum = wpool.tile([128, 1], dt.float32, tag="ssum")
                nc.scalar.activation(
                    esc[:], lps[:], mybir.ActivationFunctionType.Exp,
                    bias=mneg[:], accum_out=ssum[:],
                )
                lse = wpool.tile([128, 1], dt.float32, tag="lse")
                nc.scalar.activation(lse[:], ssum[:],
                                     mybir.ActivationFunctionType.Ln)
                shift = wpool.tile([128, 1], dt.float32, tag="shift")
                nc.vector.tensor_tensor(shift[:], mneg[:], lse[:],
                                        op=mybir.AluOpType.subtract)
                osb = wpool.tile([128, C], dt.float32, tag="osb")
                nc.vector.tensor_scalar_add(osb[:], lps[:], shift[:])
                nc.sync.dma_start(out_d[b * 128:(b + 1) * 128, :], osb[:])

            def body():
                # phase 1 lo: h_lo complete after 24 batches
                for gi in range(LOBLK // GB):
                    phase1(gi)
                # phase 1 hi + (scheduler-overlapped) lo gathers/aggs
                for gi in range(LOBLK // GB, NGRP1):
                    phase1(gi)
                for b in range(NB):
                    xtp = agg_pass(b, T_LO, iota_lo, dllo_t, ixlo_t,
                                   h_lo_d, lo=True)
                    nc.vector.tensor_copy(xlo_all[:, b, :], xtp[:])
                for b in range(NB):
                    xtp = agg_pass(b, T_HI, iota_hi, dlhi_t, ixhi_t,
                                   h_hi_d, lo=False)
                    head(b, xtp)

            if bench_R:
                with tc.For_i(0, bench_R, 1):
                    body()
            else:
                body()

    nc.compile()
    return nc


def _balance_groups(deg_lo, deg_hi):
    """Greedily assign nodes to NG groups (<=128 each), jointly balancing
    lo- and hi-edge sums (normalized by their global masses, which differ
    when the lo/hi table split is asymmetric).

    Returns (grp_of, slot_of) int32 arrays of length N."""
    w_lo = NG / max(deg_lo.sum(), 1.0)
    w_hi = NG / max(deg_hi.sum(), 1.0)
    order = np.argsort(-(deg_lo + deg_hi), kind="stable")
    lo_s = np.zeros(NG, np.float64)
    hi_s = np.zeros(NG, np.float64)
    sizes = np.zeros(NG, np.int64)
    grp_of = np.empty(N, np.int32)
    slot_of = np.empty(N, np.int32)
    full = np.zeros(NG, bool)
    BIG = 1e18
    for n in order.tolist():
        cost = np.maximum((lo_s + deg_lo[n]) * w_lo,
                          (hi_s + deg_hi[n]) * w_hi)
        cost[full] = BIG
        g = int(np.argmin(cost))
        grp_of[n] = g
        slot_of[n] = sizes[g]
        sizes[g] += 1
        if sizes[g] == 128:
            full[g] = True
        lo_s[g] += deg_lo[n]
        hi_s[g] += deg_hi[n]
    return grp_of, slot_of


def _pack_idx(vals):
    """[T*128] int16 -> [128, T*8] device idx layout.

    dma_gather reads idx i from [i % 16, i // 16] of a [16, NI/16] block,
    replicated across the 8 gpsimd core groups."""
    a = vals.reshape(-1, 16).T          # [16, NI/16]
    return np.tile(a, (8, 1))


def _prep(features, W_conv, b_conv, W2, b2, src, dst):
    import ml_dtypes
    bf16 = ml_dtypes.bfloat16

    src = np.asarray(src).astype(np.int64)
    dst = np.asarray(dst).astype(np.int64)
    deg_out = np.bincount(src, minlength=N).astype(np.float32)
    deg_in = np.bincount(dst, minlength=N).astype(np.float32)
    norm_src = 1.0 / np.sqrt(deg_out)
    norm_dst = 1.0 / np.sqrt(deg_in)

    # h-table row of node s: (s%128)*NBLKS + s//128 - base; h_lo holds node
    # blocks < LOBLK, h_hi the rest
    s_nb = src // 128
    s_p = src % 128
    elo = s_nb < LOBLK
    s_row = np.where(elo, s_p * LOBLK + s_nb, s_p * HIBLK + (s_nb - LOBLK))

    # per-dst lo/hi in-degrees for the 2D balance
    dlo = np.bincount(dst[elo], minlength=N).astype(np.float32)
    dhi = deg_in - dlo
    grp_of, slot_of = _balance_groups(dlo, dhi)

    # normalized features, transposed + blocked for phase 1:
    # featT[p, (nb*4 + c)*128 + j] = Xn[nb*128 + j, c*128 + p]
    feat_n = (np.asarray(features, np.float32) * norm_src[:, None]).astype(bf16)
    xp = np.zeros((NPAD, F), bf16)
    xp[:N] = feat_n
    featT = np.ascontiguousarray(
        xp.reshape(NBLK, 128, 4, 128).transpose(3, 0, 2, 1)
    ).reshape(128, NBLK * 512)

    # edges -> (group, lo/hi, tile, lane)
    eg = grp_of[dst]
    cnt_lo = np.bincount(eg[elo], minlength=NG)
    cnt_hi = np.bincount(eg[~elo], minlength=NG)
    T_LO = int(np.ceil(cnt_lo.max() / 128))
    T_HI = int(np.ceil(cnt_hi.max() / 128))

    dloc = slot_of[dst].astype(np.float32)

    def layout(mask, cnts, L):
        r_e = s_row[mask]
        g_e = eg[mask]
        d_e = dloc[mask]
        # sort by (group, table row) so each tile's rows ascend in the table
        order = np.lexsort((r_e, g_e))
        g_s = g_e[order]
        starts = np.zeros(NG + 1, np.int64)
        np.cumsum(cnts, out=starts[1:])
        pos = np.arange(len(g_s)) - starts[g_s]
        slot = g_s * L + pos
        idx_pad = np.zeros(NG * L, np.int16)
        dl_pad = np.full(NG * L, 255.0, np.float32)
        idx_pad[slot] = r_e[order].astype(np.int16)
        dl_pad[slot] = d_e[order]
        return idx_pad.reshape(NG, L), dl_pad.reshape(NG, L)

    idx_lo, dl_lo = layout(elo, cnt_lo, T_LO * 128)
    idx_hi, dl_hi = layout(~elo, cnt_hi, T_HI * 128)

    # normdst per core: [NB*128] values in permuted slot order, pad 1.0
    nd = np.ones(NG * 128, np.float32)
    nd[grp_of * 128 + slot_of] = norm_dst

    iota = np.broadcast_to(np.arange(128, dtype=np.float32), (128, 128)).astype(bf16)
    wconv = np.ascontiguousarray(
        np.asarray(W_conv, np.float32).reshape(4, 128, H).transpose(1, 0, 2)
    ).reshape(128, 4 * H).astype(bf16)
    w2r = np.ascontiguousarray(
        np.asarray(W2, np.float32).reshape(2, 128, C).transpose(1, 0, 2)
    ).reshape(128, 2 * C).astype(bf16)

    in_maps = []
    for c in range(P):
        gsl = slice(c * NB, (c + 1) * NB)
        # dstloc per core+split: [128, NB*T_X]; tile column (b*T_X + t),
        # lane p holds the slot of edge (tile t of block b, lane p)
        dllo = np.ascontiguousarray(
            dl_lo[gsl].reshape(NB * T_LO, 128).T).astype(bf16)
        dlhi = np.ascontiguousarray(
            dl_hi[gsl].reshape(NB * T_HI, 128).T).astype(bf16)
        ixlo = np.concatenate(
            [_pack_idx(idx_lo[g]) for g in range(c * NB, (c + 1) * NB)],
            axis=1)
        ixhi = np.concatenate(
            [_pack_idx(idx_hi[g]) for g in range(c * NB, (c + 1) * NB)],
            axis=1)
        in_maps.append({
            "featT": featT,
            "ixlo": np.ascontiguousarray(ixlo),
            "ixhi": np.ascontiguousarray(ixhi),
            "dllo": dllo,
            "dlhi": dlhi,
            "normdst": np.ascontiguousarray(np.broadcast_to(
                nd[c * NPC:(c + 1) * NPC].astype(bf16), (128, NB * 128))),
            "iota": iota,
            "wconv": wconv,
            "w2": w2r,
            "ones1": np.ones((1, 128), np.float32).astype(bf16),
            "bconv": np.asarray(b_conv, np.float32).reshape(2, 128).T.copy(),
            "b2r": np.asarray(b2, np.float32).reshape(1, C).astype(bf16),
        })
    return T_LO, T_HI, grp_of, slot_of, in_maps


def kernel(features, W_conv, b_conv, W2, b2, src, dst):
    from concourse.bass_utils import run_bass_kernel_spmd

    T_LO, T_HI, grp_of, slot_of, in_maps = _prep(
        features, W_conv, b_conv, W2, b2, src, dst)
    key = (T_LO, T_HI)
    if key not in _cache:
        _cache[key] = _build_program(T_LO, T_HI)
    nc = _cache[key]
    res = run_bass_kernel_spmd(nc, in_maps, core_ids=list(range(P)),
                               trace=_trace, tmpdir=_trace_tmpdir)
    global _last_results
    _last_results = res
    rows = np.concatenate([res.results[c]["out"] for c in range(P)], axis=0)
    out = rows[grp_of * 128 + slot_of]
    return out.astype(np.float32)
